# revision 79
# baseline (speedup 1.0000x reference)
"""MLA (multi-head latent attention) Bass kernel for TRN2, 8-core SPMD.

Sharding: DP over batch (2) x TP over heads (4 groups of 4 heads).
core c: batch b = c // 4, head-group g = c % 4 (heads 4g..4g+3).

Math (per core):
  q_heads = (x @ Wq_eff_g) / rms_q          Wq_eff = Wqa @ diag(qw) @ Wqb  (host-folded)
  rms_q   = sqrt(mean_r((x @ Wqa)_r^2) + eps)   ssq via per-core 384-col slice + AllReduce
  kv_a    = x @ Wkva ; kv_latent = kv_a[:, :512] ; k_rope = rope(kv_a[:, 512:])
  kv_norm = kv_latent / rms_kv
  k_nope  = kv_norm @ Wkvb_k_g ; v = kv_norm @ Wkvb_v_g
  e[k,q]  = exp(SCALE * (qT . kT)) * tri-mask    (transposed scores, no max-sub)
  attnT   = (v^T e) / (1^T e)                    per head
  outT    = Wout_g^T @ attnT                     partial over heads, host sums (bf16)
"""

import hashlib
from contextlib import ExitStack
import numpy as np
import ml_dtypes

import concourse.bass as bass
import concourse.mybir as mybir
import concourse.tile as tile
from concourse.masks import make_identity

F32 = mybir.dt.float32
F32R = mybir.dt.float32r
BF16 = mybir.dt.bfloat16
AF = mybir.ActivationFunctionType

B, S, D = 2, 1024, 2048
H, DN, DR, DV = 16, 128, 64, 128
RQ, RKV = 1536, 512
THETA = 10000.0
EPS = 1e-6
SCALE = float((DN + DR) ** -0.5)

NCORE = 8
TP = 4                  # head groups
HPG = H // TP           # 4 heads per core
NT = S // 128           # 8 token blocks
NSP = S // 512          # 2 q spans
KC = D // 128           # 16 contraction chunks over D
RC = RKV // 128         # 4 contraction chunks over RKV
WQA_SL = RQ // TP       # 384 per-core Wqa column slice (for ssq)

SKIP, FREE, MIXED = 0, 1, 2


def _chunked(ap, n):
    """Views [128, n*w] tile as n chunks [128, w]."""
    w = ap.shape[1] // n
    return [ap[:, i * w:(i + 1) * w] for i in range(n)]


def build_program(block_cls, n_mixed=None, use_collective=True,
                  wqa_cols=WQA_SL, trn_type="TRN2", fix_waits=True, reps=1,
                  use_kv_ag=False):
    """block_cls is kept for interface compat; the program assumes the
    causal block structure that prep_core_inputs asserts."""
    nc = bass.Bass(trn_type, num_devices=NCORE if use_collective else 1)
    use_kv_ag = use_kv_ag and use_collective

    # all inputs host-packed as [128, (rows/128)*cols] so each is ONE flat DMA
    xT = nc.dram_tensor("xT", [128, KC * S], BF16, kind="ExternalInput")
    xTkv = nc.dram_tensor("xTkv", [128, KC * (S // TP)], BF16,
                          kind="ExternalInput")
    wqa = nc.dram_tensor("wqa", [128, KC * wqa_cols], BF16, kind="ExternalInput")
    wqn = nc.dram_tensor("wqn", [128, KC * HPG * DN], BF16, kind="ExternalInput")
    wqr = nc.dram_tensor("wqr", [128, KC * HPG * DR], BF16, kind="ExternalInput")
    wkva = nc.dram_tensor("wkva", [128, KC * (RKV + DR)], BF16,
                          kind="ExternalInput")
    wkbk = nc.dram_tensor("wkbk", [128, RC * HPG * DN], BF16,
                          kind="ExternalInput")
    wkbv = nc.dram_tensor("wkbv", [128, RC * HPG * DV], BF16,
                          kind="ExternalInput")
    wout = nc.dram_tensor("wout", [128, HPG * D], BF16, kind="ExternalInput")
    cost2 = nc.dram_tensor("cost2", [128, S], BF16, kind="ExternalInput")
    sint2 = nc.dram_tensor("sint2", [128, S], BF16, kind="ExternalInput")
    cosk = nc.dram_tensor("cosk", [128, NT * DR], BF16, kind="ExternalInput")
    sink = nc.dram_tensor("sink", [128, NT * DR], BF16, kind="ExternalInput")
    perm = nc.dram_tensor("perm", [128, 128], BF16, kind="ExternalInput")
    masks = nc.dram_tensor("masks", [128, 128], BF16, kind="ExternalInput")
    outT = nc.dram_tensor("outT", [128, KC * S], BF16, kind="ExternalOutput")

    with tile.TileContext(nc) as tc:
        for _rep in range(reps):
            with ExitStack() as ctx:
                _emit(ctx, nc, tc, locals(), use_collective, wqa_cols,
                      use_kv_ag)
    if fix_waits:
        _fix_multiwait(nc)
    return nc


def _emit(ctx, nc, tc, t, use_collective, wqa_cols, use_kv_ag=False):
    xT, wqa, wqn, wqr, wkva, wkbk, wkbv, wout = (
        t["xT"], t["wqa"], t["wqn"], t["wqr"], t["wkva"], t["wkbk"],
        t["wkbv"], t["wout"])
    xTkv = t["xTkv"]
    cost2, sint2, cosk, sink = t["cost2"], t["sint2"], t["cosk"], t["sink"]
    perm, masks, outT = t["perm"], t["masks"], t["outT"]

    # ---------------- persistent pools + all input DMAs up front ------------
    p_in = ctx.enter_context(tc.tile_pool(name="p_in", bufs=1))
    p_mid = ctx.enter_context(tc.tile_pool(name="p_mid", bufs=1))
    p_dram = ctx.enter_context(tc.tile_pool(name="p_dram", bufs=1, space="DRAM"))

    def load(dram, nchunk, tag):
        sb = p_in.tile([128, dram.shape[1]], BF16, tag=tag, name=tag)
        nc.sync.dma_start(sb[:], dram[:, :])
        return _chunked(sb, nchunk)

    # x streamed in token pieces so phase L's first block starts as soon as
    # tokens 0-127 and wqa have landed
    x_tile = p_in.tile([128, KC * S], BF16, tag="xt", name="xt")

    def load_x_toks(t0, t1):
        ts = slice(t0, t1)
        nc.sync.dma_start(
            x_tile[:].rearrange("p (c s) -> p c s", c=KC)[:, :, ts],
            xT[:, :].rearrange("p (c s) -> p c s", c=KC)[:, :, ts])

    load_x_toks(0, 128)
    x_sb = _chunked(x_tile, KC)
    wqa_sb = load(wqa, KC, "wqa")
    load_x_toks(128, 512)
    load_x_toks(512, S)
    wkva_sb = load(wkva, KC, "wkva")
    xtkv_sb = load(xTkv, KC, "xtkv") if use_kv_ag else None
    wqn_sb = load(wqn, KC, "wqn")
    wqr_sb = load(wqr, KC, "wqr")
    cosk_sb = load(cosk, 1, "cosk")[0]
    sink_sb = load(sink, 1, "sink")[0]
    perm_sb = load(perm, 1, "perm")[0]
    tri_sb = load(masks, 1, "tri")[0]

    ident = p_in.tile([128, 128], BF16, tag="ident")
    make_identity(nc, ident[:])
    ones = p_in.tile([128, 1], BF16, tag="ones")
    nc.gpsimd.memset(ones[:], 1.0)
    epsc = p_in.tile([128, 1], F32, tag="epsc")
    nc.gpsimd.memset(epsc[:], EPS)
    ones_f = p_in.tile([1, 128], F32, tag="ones_f")
    nc.gpsimd.memset(ones_f[:], 1.0)
    ones_r = p_in.tile([1, 128], F32R, tag="ones_r")
    with nc.allow_low_precision(reason="f32r broadcast operand"):
        nc.scalar.copy(ones_r[:], ones_f[:])

    # ---------------- phase L: q ssq (partial cols) + AllReduce -------------
    ssq_in = p_dram.tile([S], F32)
    ssq_out = p_dram.tile([S], F32)
    with tc.tile_pool(name="lp", bufs=2, space="PSUM") as lp, \
         tc.tile_pool(name="ls", bufs=2) as ls:
        for tb in range(NT):
            ps = lp.tile([128, wqa_cols], F32, tag="qa")
            for kc in range(KC):
                nc.tensor.matmul(ps[:], x_sb[kc][:, tb * 128:(tb + 1) * 128],
                                 wqa_sb[kc][:], start=(kc == 0),
                                 stop=(kc == KC - 1))
            scr = ls.tile([128, wqa_cols], BF16, tag="scr")
            # bufs=NT: the SP DMA queue is busy with input loads; don't let
            # ssq slot reuse stall the Squares (and thus the L matmuls)
            ssq = ls.tile([128, 1], F32, tag="ssq", bufs=NT)
            nc.scalar.activation(scr[:], ps[:], AF.Square, accum_out=ssq[:])
            nc.sync.dma_start(ssq_in[tb * 128:(tb + 1) * 128], ssq[:, 0])
    if use_collective:
        nc.gpsimd.collective_compute(
            "AllReduce", mybir.AluOpType.add,
            replica_groups=[[0, 1, 2, 3], [4, 5, 6, 7]],
            ins=[ssq_in.opt()], outs=[ssq_out.opt()])
    else:
        nc.sync.dma_start(ssq_out[:], ssq_in[:])

    # ------- sharded kv_a: this core's S/TP tokens, then AllGather ----------
    kvag_out = None
    if use_kv_ag:
        kvag_in = p_dram.tile([S // TP, RKV + DR], BF16, name="kvag_in")
        kvag_out = p_dram.tile([S, RKV + DR], BF16, name="kvag_out")
        with tc.tile_pool(name="gp", bufs=2, space="PSUM") as gp, \
             tc.tile_pool(name="gp2", bufs=2, space="PSUM") as gp2, \
             tc.tile_pool(name="gs", bufs=2) as gs:
            for tb2 in range(S // TP // 128):
                ps = gp.tile([128, RKV], F32, tag="kva")
                pr = gp2.tile([128, DR], F32, tag="kvr")
                for kc in range(KC):
                    nc.tensor.matmul(
                        ps[:], xtkv_sb[kc][:, tb2 * 128:(tb2 + 1) * 128],
                        wkva_sb[kc][:, :RKV], start=(kc == 0),
                        stop=(kc == KC - 1))
                for kc in range(KC):
                    nc.tensor.matmul(
                        pr[:], xtkv_sb[kc][:, tb2 * 128:(tb2 + 1) * 128],
                        wkva_sb[kc][:, RKV:], start=(kc == 0),
                        stop=(kc == KC - 1))
                st = gs.tile([128, RKV + DR], BF16, tag="kvst")
                nc.scalar.copy(st[:, :RKV], ps[:])
                nc.vector.tensor_copy(st[:, RKV:], pr[:])
                nc.sync.dma_start(kvag_in[tb2 * 128:(tb2 + 1) * 128, :], st[:])
        nc.gpsimd.collective_compute(
            "AllGather", mybir.AluOpType.bypass,
            replica_groups=[[0, 1, 2, 3], [4, 5, 6, 7]],
            ins=[kvag_in.opt()], outs=[kvag_out.opt()])

    # read the AllReduce result back immediately (deferred loads may queue
    # behind it — they aren't needed until Q/KB/O)
    p_rq = ctx.enter_context(tc.tile_pool(name="p_rq", bufs=1))
    r1 = p_rq.tile([1, S], F32, tag="r1")
    nc.sync.dma_start(r1[:], ssq_out[:].rearrange("(one s) -> one s", one=1))

    wkbk_sb = load(wkbk, RC, "wkbk")
    wkbv_sb = load(wkbv, RC, "wkbv")
    cost2_sb = load(cost2, 1, "cost2")[0]
    sint2_sb = load(sint2, 1, "sint2")[0]
    wout_sb = load(wout, HPG, "wout")

    # ---------------- phase KV: kv_a, rmsnorm, rope(k), transposes ----------
    # No Pool-engine work here: the AllReduce above occupies the Pool queue
    # and would stall drains behind the inter-core barrier.
    # k-rope kept in TWO half-padded d-major tiles so the score matmuls can
    # pair head-even (qr rows 0-63) / head-odd (rows 64-127) operands with a
    # common base partition of 0.
    krT_lo = p_mid.tile([128, S], BF16, tag="krTl")
    krT_hi = p_mid.tile([128, S], BF16, tag="krTh")
    kvnT = [p_mid.tile([128, S], BF16, tag=f"kvn{rc}", name=f"kvn{rc}")
            for rc in range(RC)]
    # double-buffered [tok, 128] staging for kr with permanently-zero halves
    kr_lo = [p_mid.tile([128, 128], BF16, tag=f"krlo{i}", name=f"krlo{i}")
             for i in range(2)]
    kr_hi = [p_mid.tile([128, 128], BF16, tag=f"krhi{i}", name=f"krhi{i}")
             for i in range(2)]
    for i in range(2):
        nc.vector.memset(kr_lo[i][:, 64:], 0.0)
        nc.vector.memset(kr_hi[i][:, :64], 0.0)
    rq_bc = p_mid.tile([128, S], F32, tag="rq")
    p_rqs = ctx.enter_context(tc.tile_pool(name="p_rqs", bufs=1))

    def emit_rq_chain(rqp):
        # broadcast 1/rms_q over partitions; emitted mid-KV so its Act/DVE
        # ops land mid-queue and rq_bc is ready before phase Q's drains
        r2 = p_rqs.tile([1, S], F32, tag="r2")
        nc.scalar.activation(r2[:], r1[:], AF.Sqrt, scale=1.0 / RQ,
                             bias=epsc[0:1, 0:1])
        r3 = p_rqs.tile([1, S], F32R, tag="r3")
        with nc.allow_low_precision(reason="f32r broadcast operand"):
            nc.vector.reciprocal(r3[:], r2[:])
        for sp in range(NSP):
            pb = rqp.tile([128, 512], F32, tag="pb")
            nc.tensor.matmul(pb[:], ones_r[:], r3[:, sp * 512:(sp + 1) * 512],
                             start=True, stop=True)
            nc.scalar.copy(rq_bc[:, sp * 512:(sp + 1) * 512], pb[:])

    with tc.tile_pool(name="kp", bufs=2, space="PSUM") as kp, \
         tc.tile_pool(name="kp2", bufs=2, space="PSUM") as kp2, \
         tc.tile_pool(name="kpt", bufs=2, space="PSUM") as kpt, \
         tc.tile_pool(name="rqp", bufs=2, space="PSUM") as rqp, \
         tc.tile_pool(name="kin", bufs=3) as kin, \
         tc.tile_pool(name="ks", bufs=2) as ks:
        for tb in range(NT):
            if tb == NT // 2:
                emit_rq_chain(rqp)
            if use_kv_ag:
                kva = kin.tile([128, RKV + DR], BF16, tag="kva")
                nc.sync.dma_start(kva[:], kvag_out[tb * 128:(tb + 1) * 128, :])
                ps, pr = kva[:, :RKV], kva[:, RKV:]
            else:
                ps = kp.tile([128, RKV], F32, tag="kva")
                pr = kp2.tile([128, DR], F32, tag="kvr")
                for kc in range(KC):
                    nc.tensor.matmul(ps[:],
                                     x_sb[kc][:, tb * 128:(tb + 1) * 128],
                                     wkva_sb[kc][:, :RKV], start=(kc == 0),
                                     stop=(kc == KC - 1))
                for kc in range(KC):
                    nc.tensor.matmul(pr[:],
                                     x_sb[kc][:, tb * 128:(tb + 1) * 128],
                                     wkva_sb[kc][:, RKV:], start=(kc == 0),
                                     stop=(kc == KC - 1))
            # rmsnorm stats straight off PSUM (or the gathered tile)
            scr = ks.tile([128, RKV], BF16, tag="scr")
            ssq = ks.tile([128, 1], F32, tag="ssq")
            nc.scalar.activation(scr[:], ps[:], AF.Square, accum_out=ssq[:])
            rk = ks.tile([128, 1], F32, tag="rk")
            nc.scalar.activation(rk[:], ssq[:], AF.Sqrt, scale=1.0 / RKV,
                                 bias=epsc[:, 0:1])
            nc.vector.reciprocal(ssq[:], rk[:])
            kvn = ks.tile([128, RKV], BF16, tag="kvn")
            nc.scalar.activation(kvn[:], ps[:], AF.Copy, scale=ssq[:, 0:1])
            # rope on k_rope (token-major)
            co = cosk_sb[:, tb * DR:(tb + 1) * DR]
            si = sink_sb[:, tb * DR:(tb + 1) * DR]
            t1 = ks.tile([128, DR], F32, tag="t1")
            nc.vector.tensor_mul(t1[:, 0::2], pr[:, 1::2], si[:, 0::2])
            nc.vector.tensor_mul(t1[:, 1::2], pr[:, 0::2], si[:, 1::2])
            t2 = ks.tile([128, DR], F32, tag="t2")
            nc.vector.tensor_mul(t2[:], pr[:], co)
            klo, khi = kr_lo[tb % 2], kr_hi[tb % 2]
            nc.vector.tensor_add(klo[:, :64], t1[:], t2[:])
            nc.vector.tensor_add(khi[:, 64:], t1[:], t2[:])
            # transposes (drained on Act/DVE; Pool has no PSUM port)
            ptl = kpt.tile([128, 128], BF16, tag="pt")
            nc.tensor.transpose(ptl[:], klo[:], ident[:])
            nc.scalar.copy(krT_lo[:, tb * 128:(tb + 1) * 128], ptl[:])
            pth = kpt.tile([128, 128], BF16, tag="pt")
            nc.tensor.transpose(pth[:], khi[:], ident[:])
            nc.vector.tensor_copy(krT_hi[:, tb * 128:(tb + 1) * 128], pth[:])
            for rc in range(RC):
                pt2 = kpt.tile([128, 128], BF16, tag="pt")
                nc.tensor.transpose(pt2[:], kvn[:, rc * 128:(rc + 1) * 128],
                                    ident[:])
                if rc % 2 == 0:
                    nc.scalar.copy(kvnT[rc][:, tb * 128:(tb + 1) * 128], pt2[:])
                else:
                    nc.vector.tensor_copy(kvnT[rc][:, tb * 128:(tb + 1) * 128],
                                          pt2[:])

    # ---------------- phase Q: q_nope + q_rope (both d-major) ---------------
    # rms scaling fused into the PSUM drains via rq_bc.
    qnS = [p_mid.tile([128, S], BF16, tag=f"qnS{h}", name=f"qnS{h}")
           for h in range(HPG)]
    qrS = [p_mid.tile([128, S], BF16, tag=f"qrS{hp}", name=f"qrS{hp}")
           for hp in range(HPG // 2)]
    with tc.tile_pool(name="qp", bufs=2, space="PSUM") as qp, \
         tc.tile_pool(name="qp2", bufs=2, space="PSUM") as qp2, \
         tc.tile_pool(name="qs", bufs=3) as qs:
        for h in range(HPG):
            for sp in range(NSP):
                ps = qp.tile([128, 512], F32, tag="qn")
                for kc in range(KC):
                    nc.tensor.matmul(
                        ps[:], wqn_sb[kc][:, h * DN:(h + 1) * DN],
                        x_sb[kc][:, sp * 512:(sp + 1) * 512],
                        start=(kc == 0), stop=(kc == KC - 1))
                nc.vector.tensor_mul(qnS[h][:, sp * 512:(sp + 1) * 512],
                                     ps[:], rq_bc[:, sp * 512:(sp + 1) * 512])
        # q_rope: head pairs stacked on partitions (rows 0-63 / 64-127)
        for hp in range(HPG // 2):
            for sp in range(NSP):
                sl = slice(sp * 512, (sp + 1) * 512)
                ps = qp.tile([128, 512], F32, tag="qr")
                for kc in range(KC):
                    nc.tensor.matmul(
                        ps[:], wqr_sb[kc][:, hp * 128:(hp + 1) * 128],
                        x_sb[kc][:, sl], start=(kc == 0), stop=(kc == KC - 1))
                # scale by 1/rms first (commutes with rope), then rotate
                qr0 = qs.tile([128, 512], BF16, tag="qr0")
                nc.vector.tensor_mul(qr0[:], ps[:], rq_bc[:, sl])
                ps2 = qp2.tile([128, 512], F32, tag="shuf")
                nc.tensor.matmul(ps2[:], perm_sb[:], qr0[:],
                                 start=True, stop=True)
                t1 = qs.tile([128, 512], F32, tag="t1")
                nc.vector.tensor_mul(t1[:], qr0[:], cost2_sb[:, sl])
                t2 = qs.tile([128, 512], F32, tag="t2")
                nc.vector.tensor_mul(t2[:], ps2[:], sint2_sb[:, sl])
                nc.vector.tensor_add(qrS[hp][:, sl], t1[:], t2[:])

    # ---------------- phase KB: k_nope (d-major) + v (token-major) ----------
    knT = [p_mid.tile([128, S], BF16, tag=f"kn{h}", name=f"kn{h}")
           for h in range(HPG)]
    v_sb = [p_mid.tile([128, HPG * DV], BF16, tag=f"v{tb}", name=f"v{tb}")
            for tb in range(NT)]
    with tc.tile_pool(name="bp", bufs=2, space="PSUM") as bp:
        for h in range(HPG):
            for sp in range(NSP):
                ps = bp.tile([128, 512], F32, tag="kn")
                for rc in range(RC):
                    nc.tensor.matmul(
                        ps[:], wkbk_sb[rc][:, h * DN:(h + 1) * DN],
                        kvnT[rc][:, sp * 512:(sp + 1) * 512],
                        start=(rc == 0), stop=(rc == RC - 1))
                if (h + sp) % 2 == 0:
                    nc.scalar.copy(knT[h][:, sp * 512:(sp + 1) * 512], ps[:])
                else:
                    nc.vector.tensor_copy(knT[h][:, sp * 512:(sp + 1) * 512],
                                          ps[:])
        for tb in range(NT):
            ps = bp.tile([128, HPG * DV], F32, tag="v")
            for rc in range(RC):
                nc.tensor.matmul(ps[:], kvnT[rc][:, tb * 128:(tb + 1) * 128],
                                 wkbv_sb[rc][:], start=(rc == 0),
                                 stop=(rc == RC - 1))
            if tb % 2 == 0:
                nc.scalar.copy(v_sb[tb][:], ps[:])
            else:
                nc.vector.tensor_copy(v_sb[tb][:], ps[:])

    # ------- phase A+O: attention (transposed flash, exact widths) + out ----
    attnT = [p_mid.tile([128, S], BF16, tag=f"at{h}", name=f"at{h}")
             for h in range(HPG)]
    with tc.tile_pool(name="op", bufs=2, space="PSUM") as op_, \
         tc.tile_pool(name="ap", bufs=2, space="PSUM") as ap_, \
         tc.tile_pool(name="sp", bufs=2, space="PSUM") as sp_, \
         tc.tile_pool(name="dp", bufs=2, space="PSUM") as dp, \
         tc.tile_pool(name="as_", bufs=3) as as_, \
         tc.tile_pool(name="os_", bufs=4) as os_:
        for sp in range(NSP):
            q0 = sp * 512
            nkb = 4 * (sp + 1)
            for h in range(HPG):
                acc = ap_.tile([128, 512], F32, tag="acc")
                den = dp.tile([1, 512], F32, tag="dn")
                for kb in range(nkb):
                    off = max(0, kb * 128 - q0)   # first visible col in span
                    ps = sp_.tile([128, 512], F32, tag="s")
                    nc.tensor.matmul(ps[:, off:],
                                     knT[h][:, kb * 128:(kb + 1) * 128],
                                     qnS[h][:, q0 + off:q0 + 512],
                                     start=True, stop=False)
                    krT_h = krT_lo if h % 2 == 0 else krT_hi
                    nc.tensor.matmul(ps[:, off:],
                                     krT_h[:, kb * 128:(kb + 1) * 128],
                                     qrS[h // 2][:, q0 + off:q0 + 512],
                                     start=False, stop=True)
                    e = as_.tile([128, 512], BF16, tag="e")
                    nc.scalar.activation(e[:, off:], ps[:, off:], AF.Exp,
                                         scale=SCALE)
                    if kb * 128 >= q0:
                        # diagonal block: triangle on the first visible 128
                        nc.vector.tensor_mul(e[:, off:off + 128],
                                             e[:, off:off + 128], tri_sb[:])
                    # exact-width accumulation: cols [0,off) finished their
                    # sums in earlier kb blocks; start zeroes the whole bank
                    # at kb=0 (off=0) and stop ends the bank's group
                    st, fin = (kb == 0), (kb == nkb - 1)
                    nc.tensor.matmul(acc[:, off:],
                                     v_sb[kb][:, h * DV:(h + 1) * DV],
                                     e[:, off:], start=st, stop=fin)
                    nc.tensor.matmul(den[:, off:], ones[:, 0:1], e[:, off:],
                                     start=st, stop=fin)
                rd = as_.tile([1, 512], F32R, tag="rd", bufs=2)
                with nc.allow_low_precision(reason="f32r broadcast operand"):
                    nc.vector.reciprocal(rd[:], den[:])
                rdp = dp.tile([128, 512], F32, tag="dn")
                nc.tensor.matmul(rdp[:], ones_r[:], rd[:], start=True,
                                 stop=True)
                rdb = as_.tile([128, 512], F32, tag="rdb", bufs=2)
                nc.vector.tensor_copy(rdb[:], rdp[:])
                nc.vector.tensor_mul(attnT[h][:, q0:q0 + 512], acc[:], rdb[:])
            # output projection for this span
            for mb in range(KC):
                po = op_.tile([128, 512], F32, tag="o")
                for hc in range(HPG):
                    nc.tensor.matmul(po[:],
                                     wout_sb[hc][:, mb * 128:(mb + 1) * 128],
                                     attnT[hc][:, q0:q0 + 512],
                                     start=(hc == 0), stop=(hc == HPG - 1))
                ot = os_.tile([128, 512], BF16, tag="ot")
                # alternate drain engines; the tail of the last span goes to
                # Act, which has drained its exp queue by then
                if mb % 2 == 0 or (sp == NSP - 1 and mb >= KC - 2):
                    nc.scalar.copy(ot[:], po[:])
                else:
                    nc.vector.tensor_copy(ot[:], po[:])
                # alternate output DMAs across the Pool and SP queues: one
                # queue alone can't sustain PE's drain production rate
                dma_eng = nc.gpsimd if mb % 2 == 0 else nc.sync
                dma_eng.dma_start(outT[:, mb * S + q0:mb * S + q0 + 512],
                                  ot[:])


def _fix_multiwait(nc):
    """This container's walrus only supports ONE sem-wait per instruction.
    Hoist excess waits onto freshly inserted same-engine Drain instructions
    placed immediately before the owner (engine executes in order, so the
    AND-semantics of multiple waits is preserved)."""
    import bass_rust
    n = [0]
    for fn in nc.m.functions:
        for blk in fn.blocks:
            out, changed = [], False
            for inst in blk.instructions:
                si = inst.sync_info
                waits = list(si.on_wait) if (si is not None and si.on_wait) else []
                if len(waits) > 1:
                    changed = True
                    for w in waits[:-1]:
                        n[0] += 1
                        d = bass_rust.InstDrain(
                            name=f"MWFIX-{n[0]}", engine=inst.engine,
                            ins=[], outs=[])
                        d.sync_info = bass_rust.SyncInfo(on_wait=[w],
                                                         on_update=[])
                        out.append(d)
                    si.on_wait = [waits[-1]]
                    inst.sync_info = si
                out.append(inst)
            if changed:
                blk.instructions = out


# ======================= host-side preparation =======================

def _bf16(a):
    return np.asarray(a, np.float32).astype(ml_dtypes.bfloat16)


def rope_tables():
    inv_freq = 1.0 / THETA ** (np.arange(0, DR, 2, dtype=np.float32) / DR)
    pos = np.arange(S, dtype=np.float32)
    freqs = np.outer(pos, inv_freq)
    emb = np.concatenate([freqs, freqs], axis=-1)          # [S, 64]
    cos = np.cos(emb).astype(np.float32)
    sin = np.sin(emb).astype(np.float32)
    sin_s = sin.copy()
    sin_s[:, 0::2] *= -1.0
    return cos, sin_s


def analyze_mask(mask):
    """mask: [1,1,S,S] additive causal. Returns block_cls (for the program
    cache key / interface compat) + the shared [128,128] triangular tile."""
    m = np.asarray(mask, np.float32).reshape(S, S)          # [q, k]
    block_cls = {}
    tri = None
    for qt in range(S // 256):
        for kb in range(NT):
            sub = m[qt * 256:(qt + 1) * 256, kb * 128:(kb + 1) * 128]
            if np.all(sub <= -1e8):
                block_cls[(kb, qt)] = SKIP
            elif np.all(sub == 0.0):
                block_cls[(kb, qt)] = FREE
            else:
                block_cls[(kb, qt)] = MIXED
    # the program assumes causal structure; verify and extract the diagonal
    # [k=128, q=128] visibility tile (identical for every diagonal block)
    for db in range(NT):
        sub = m[db * 128:(db + 1) * 128, db * 128:(db + 1) * 128]
        t = (sub.T > -1e8).astype(np.float32)               # [k, q]
        if tri is None:
            tri = t
        assert np.array_equal(tri, t), "non-uniform diagonal mask blocks"
    for qb in range(NT):
        for kb in range(NT):
            if kb == qb:
                continue
            sub = m[qb * 128:(qb + 1) * 128, kb * 128:(kb + 1) * 128]
            if kb < qb:
                assert np.all(sub == 0.0), "non-causal mask"
            else:
                assert np.all(sub <= -1e8), "non-causal mask"
    return block_cls, _bf16(tri)


def prep_core_inputs(inputs, wqa_cols=WQA_SL):
    """Returns (in_maps list of 8 dicts, block_cls)."""
    x = np.asarray(inputs["x"], np.float32)
    Wqa = np.asarray(inputs["Wqa"], np.float32)
    qw = np.asarray(inputs["q_a_norm_w"], np.float32)
    Wqb = np.asarray(inputs["Wqb"], np.float32)
    Wkva = np.asarray(inputs["Wkva"], np.float32)
    kvw = np.asarray(inputs["kv_a_norm_w"], np.float32)
    Wkvb = np.asarray(inputs["Wkvb"], np.float32)
    Wout = np.asarray(inputs["Wout"], np.float32)

    block_cls, tri = analyze_mask(inputs["attention_mask"])

    wq_eff = Wqa @ (qw[:, None] * Wqb)                      # [D, H*192]
    wq_eff = wq_eff.reshape(D, H, DN + DR)
    wkvb_w = kvw[:, None] * Wkvb                            # [RKV, H*256]
    wkvb_w = wkvb_w.reshape(RKV, H, DN + DV)
    wout_h = Wout.reshape(H, DV, D)

    cos, sin_s = rope_tables()                              # [S, 64]
    # token-major tables for k-rope: [128, NT*DR]
    cosk = _bf16(cos.reshape(NT, 128, DR).transpose(1, 0, 2).reshape(128, -1))
    sink = _bf16(sin_s.reshape(NT, 128, DR).transpose(1, 0, 2).reshape(128, -1))
    # dim-major stacked tables for q-rope: rows 0-63 and 64-127 identical
    cost2 = _bf16(np.concatenate([cos.T, cos.T], axis=0))   # [128, S]
    sint2 = _bf16(np.concatenate([sin_s.T, sin_s.T], axis=0))
    # pair-swap permutation (used as both rope shuffle and PE transpose aux)
    pm = np.zeros((128, 128), np.float32)
    for i in range(128):
        pm[i, i ^ 1] = 1.0
    perm = _bf16(pm)

    def pack(a):
        """[C*128, W] -> [128, C*W] chunk-packed bf16."""
        rows, w = a.shape
        c = rows // 128
        return _bf16(a.reshape(c, 128, w).transpose(1, 0, 2).reshape(128, -1))

    in_maps = []
    for c in range(NCORE):
        b, g = c // TP, c % TP
        hs = slice(g * HPG, (g + 1) * HPG)
        m = {
            "xT": pack(x[b].T),
            "xTkv": pack(x[b].T[:, g * (S // TP):(g + 1) * (S // TP)]),
            "wqa": pack(Wqa[:, g * wqa_cols:(g + 1) * wqa_cols]
                        if wqa_cols < RQ else Wqa),
            "wqn": pack(wq_eff[:, hs, :DN].reshape(D, HPG * DN)),
            "wqr": pack(wq_eff[:, hs, DN:].reshape(D, HPG * DR)),
            "wkva": pack(Wkva),
            "wkbk": pack(wkvb_w[:, hs, :DN].reshape(RKV, HPG * DN)),
            "wkbv": pack(wkvb_w[:, hs, DN:].reshape(RKV, HPG * DV)),
            "wout": pack(wout_h[hs].reshape(HPG * DV, D)),
            "cost2": cost2,
            "sint2": sint2,
            "cosk": cosk,
            "sink": sink,
            "perm": perm,
            "masks": tri,
        }
        in_maps.append(m)
    return in_maps, block_cls


def postprocess(results):
    """results: list of 8 dicts with 'outT' [128, KC*S] bf16 chunk-packed
    partials (outT[p, mb*S + s] = out^T[mb*128 + p, s])."""
    out = np.empty((B, S, D), np.float32)
    for b in range(B):
        acc = results[b * TP]["outT"].astype(np.float32)
        for g in range(1, TP):
            acc = acc + results[b * TP + g]["outT"].astype(np.float32)
        # [128, KC*S] -> [KC*128, S] = outT -> transpose to [S, D]
        acc = acc.reshape(128, KC, S).transpose(1, 0, 2).reshape(D, S)
        out[b] = acc.T
    return out


# ======================= kernel entry point =======================

_program_cache = {}


def _mask_key(block_cls, packed):
    h = hashlib.sha256()
    h.update(repr(sorted(block_cls.items())).encode())
    h.update(np.ascontiguousarray(packed).tobytes())
    return h.hexdigest()


def kernel(**inputs):
    """Full-input MLA forward on 8 NeuronCores.

    Sharding: data-parallel over batch (2) x tensor-parallel over heads
    (4 groups of 4); the per-token q-RMS statistic is AllReduce'd inside
    each batch group. Host folds Wqa@Wqb, shards weights by head, casts to
    bf16 and transposes x; device returns per-core transposed bf16 partial
    outputs which the host sums per batch group.
    """
    from concourse.bass_utils import run_bass_kernel_spmd

    in_maps, block_cls = prep_core_inputs(inputs)
    key = _mask_key(block_cls, in_maps[0]["masks"])
    nc = _program_cache.get(key)
    if nc is None:
        nc = build_program(block_cls, use_collective=True)
        _program_cache[key] = nc
    res = run_bass_kernel_spmd(nc, in_maps, core_ids=list(range(NCORE)))
    return postprocess(res.results)


# revision 87
# speedup vs baseline: 1.7780x; 1.7780x over previous
"""MLA (multi-head latent attention) Bass kernel for TRN2, 8-core SPMD.

Sharding: DP over batch (2) x TP over heads (4 groups of 4 heads).
core c: batch b = c // 4, head-group g = c % 4 (heads 4g..4g+3).

Math (per core):
  q_heads = (x @ Wq_eff_g) / rms_q          Wq_eff = Wqa @ diag(qw) @ Wqb  (host-folded)
  rms_q   = sqrt(mean_r((x @ Wqa)_r^2) + eps)   ssq via per-core 384-col slice + AllReduce
  kv_a    = x @ Wkva ; kv_latent = kv_a[:, :512] ; k_rope = rope(kv_a[:, 512:])
  kv_norm = kv_latent / rms_kv
  k_nope  = kv_norm @ Wkvb_k_g ; v = kv_norm @ Wkvb_v_g
  e[k,q]  = exp(SCALE * (qT . kT)) * tri-mask    (transposed scores, no max-sub)
  attnT   = (v^T e) / (1^T e)                    per head
  outT    = Wout_g^T @ attnT                     partial over heads, host sums (bf16)
"""

import hashlib
from contextlib import ExitStack
import numpy as np
import ml_dtypes

import concourse.bass as bass
import concourse.mybir as mybir
import concourse.tile as tile
from concourse.masks import make_identity

F32 = mybir.dt.float32
F32R = mybir.dt.float32r
BF16 = mybir.dt.bfloat16
AF = mybir.ActivationFunctionType

B, S, D = 2, 1024, 2048
H, DN, DR, DV = 16, 128, 64, 128
RQ, RKV = 1536, 512
THETA = 10000.0
EPS = 1e-6
SCALE = float((DN + DR) ** -0.5)

NCORE = 8
TP = 4                  # head groups
HPG = H // TP           # 4 heads per core
NT = S // 128           # 8 token blocks
NSP = S // 512          # 2 q spans
KC = D // 128           # 16 contraction chunks over D
RC = RKV // 128         # 4 contraction chunks over RKV
WQA_SL = RQ // TP       # 384 per-core Wqa column slice (for ssq)

SKIP, FREE, MIXED = 0, 1, 2


def _chunked(ap, n):
    """Views [128, n*w] tile as n chunks [128, w]."""
    w = ap.shape[1] // n
    return [ap[:, i * w:(i + 1) * w] for i in range(n)]


def build_program(block_cls, n_mixed=None, use_collective=True,
                  wqa_cols=WQA_SL, trn_type="TRN2", fix_waits=True, reps=1,
                  use_kv_ag=False):
    """block_cls is kept for interface compat; the program assumes the
    causal block structure that prep_core_inputs asserts."""
    nc = bass.Bass(trn_type, num_devices=NCORE if use_collective else 1)
    use_kv_ag = use_kv_ag and use_collective

    # all inputs host-packed as [128, (rows/128)*cols] so each is ONE flat DMA
    xT = nc.dram_tensor("xT", [128, KC * S], BF16, kind="ExternalInput")
    xTkv = nc.dram_tensor("xTkv", [128, KC * (S // TP)], BF16,
                          kind="ExternalInput")
    wqa = nc.dram_tensor("wqa", [128, KC * wqa_cols], BF16, kind="ExternalInput")
    wqn = nc.dram_tensor("wqn", [128, KC * HPG * DN], BF16, kind="ExternalInput")
    wqr = nc.dram_tensor("wqr", [128, KC * HPG * DR], BF16, kind="ExternalInput")
    wkva = nc.dram_tensor("wkva", [128, KC * (RKV + DR)], BF16,
                          kind="ExternalInput")
    wkbk = nc.dram_tensor("wkbk", [128, RC * HPG * DN], BF16,
                          kind="ExternalInput")
    wkbv = nc.dram_tensor("wkbv", [128, RC * HPG * DV], BF16,
                          kind="ExternalInput")
    wout = nc.dram_tensor("wout", [128, HPG * D], BF16, kind="ExternalInput")
    cost2 = nc.dram_tensor("cost2", [128, S], BF16, kind="ExternalInput")
    sint2 = nc.dram_tensor("sint2", [128, S], BF16, kind="ExternalInput")
    cosk = nc.dram_tensor("cosk", [128, NT * DR], BF16, kind="ExternalInput")
    sink = nc.dram_tensor("sink", [128, NT * DR], BF16, kind="ExternalInput")
    perm = nc.dram_tensor("perm", [128, 128], BF16, kind="ExternalInput")
    masks = nc.dram_tensor("masks", [128, 128], BF16, kind="ExternalInput")
    outT = nc.dram_tensor("outT", [128, KC * S], BF16, kind="ExternalOutput")

    with tile.TileContext(nc) as tc:
        for _rep in range(reps):
            with ExitStack() as ctx:
                _emit(ctx, nc, tc, locals(), use_collective, wqa_cols,
                      use_kv_ag)
    if fix_waits:
        _fix_multiwait(nc)
    return nc


def _emit(ctx, nc, tc, t, use_collective, wqa_cols, use_kv_ag=False):
    xT, wqa, wqn, wqr, wkva, wkbk, wkbv, wout = (
        t["xT"], t["wqa"], t["wqn"], t["wqr"], t["wkva"], t["wkbk"],
        t["wkbv"], t["wout"])
    xTkv = t["xTkv"]
    cost2, sint2, cosk, sink = t["cost2"], t["sint2"], t["cosk"], t["sink"]
    perm, masks, outT = t["perm"], t["masks"], t["outT"]

    # ---------------- persistent pools + all input DMAs up front ------------
    p_in = ctx.enter_context(tc.tile_pool(name="p_in", bufs=1))
    p_mid = ctx.enter_context(tc.tile_pool(name="p_mid", bufs=1))
    p_dram = ctx.enter_context(tc.tile_pool(name="p_dram", bufs=1, space="DRAM"))

    def load(dram, nchunk, tag):
        sb = p_in.tile([128, dram.shape[1]], BF16, tag=tag, name=tag)
        nc.sync.dma_start(sb[:], dram[:, :])
        return _chunked(sb, nchunk)

    # x streamed in token pieces so phase L's first block starts as soon as
    # tokens 0-127 and wqa have landed
    x_tile = p_in.tile([128, KC * S], BF16, tag="xt", name="xt")

    def load_x_toks(t0, t1):
        ts = slice(t0, t1)
        nc.sync.dma_start(
            x_tile[:].rearrange("p (c s) -> p c s", c=KC)[:, :, ts],
            xT[:, :].rearrange("p (c s) -> p c s", c=KC)[:, :, ts])

    load_x_toks(0, 128)
    x_sb = _chunked(x_tile, KC)
    wqa_sb = load(wqa, KC, "wqa")
    load_x_toks(128, 512)
    load_x_toks(512, S)
    wkva_sb = load(wkva, KC, "wkva")
    xtkv_sb = load(xTkv, KC, "xtkv") if use_kv_ag else None
    wqn_sb = load(wqn, KC, "wqn")
    wqr_sb = load(wqr, KC, "wqr")
    cosk_sb = load(cosk, 1, "cosk")[0]
    sink_sb = load(sink, 1, "sink")[0]
    perm_sb = load(perm, 1, "perm")[0]
    tri_sb = load(masks, 1, "tri")[0]

    ident = p_in.tile([128, 128], BF16, tag="ident")
    make_identity(nc, ident[:])
    ones = p_in.tile([128, 1], BF16, tag="ones")
    nc.gpsimd.memset(ones[:], 1.0)
    epsc = p_in.tile([128, 1], F32, tag="epsc")
    nc.gpsimd.memset(epsc[:], EPS)
    ones_f = p_in.tile([1, 128], F32, tag="ones_f")
    nc.gpsimd.memset(ones_f[:], 1.0)
    ones_r = p_in.tile([1, 128], F32R, tag="ones_r")
    with nc.allow_low_precision(reason="f32r broadcast operand"):
        nc.scalar.copy(ones_r[:], ones_f[:])

    # ---------------- phase L: q ssq (partial cols) + AllReduce -------------
    ssq_in = p_dram.tile([S], F32)
    ssq_out = p_dram.tile([S], F32)
    with tc.tile_pool(name="lp", bufs=2, space="PSUM") as lp, \
         tc.tile_pool(name="ls", bufs=2) as ls:
        for tb in range(NT):
            ps = lp.tile([128, wqa_cols], F32, tag="qa")
            for kc in range(KC):
                nc.tensor.matmul(ps[:], x_sb[kc][:, tb * 128:(tb + 1) * 128],
                                 wqa_sb[kc][:], start=(kc == 0),
                                 stop=(kc == KC - 1))
            scr = ls.tile([128, wqa_cols], BF16, tag="scr")
            # bufs=NT: the SP DMA queue is busy with input loads; don't let
            # ssq slot reuse stall the Squares (and thus the L matmuls)
            ssq = ls.tile([128, 1], F32, tag="ssq", bufs=NT)
            nc.scalar.activation(scr[:], ps[:], AF.Square, accum_out=ssq[:])
            nc.sync.dma_start(ssq_in[tb * 128:(tb + 1) * 128], ssq[:, 0])
    if use_collective:
        nc.gpsimd.collective_compute(
            "AllReduce", mybir.AluOpType.add,
            replica_groups=[[0, 1, 2, 3], [4, 5, 6, 7]],
            ins=[ssq_in.opt()], outs=[ssq_out.opt()])
    else:
        nc.sync.dma_start(ssq_out[:], ssq_in[:])

    # ------- sharded kv_a: this core's S/TP tokens, then AllGather ----------
    kvag_out = None
    if use_kv_ag:
        kvag_in = p_dram.tile([S // TP, RKV + DR], BF16, name="kvag_in")
        kvag_out = p_dram.tile([S, RKV + DR], BF16, name="kvag_out")
        with tc.tile_pool(name="gp", bufs=2, space="PSUM") as gp, \
             tc.tile_pool(name="gp2", bufs=2, space="PSUM") as gp2, \
             tc.tile_pool(name="gs", bufs=2) as gs:
            for tb2 in range(S // TP // 128):
                ps = gp.tile([128, RKV], F32, tag="kva")
                pr = gp2.tile([128, DR], F32, tag="kvr")
                for kc in range(KC):
                    nc.tensor.matmul(
                        ps[:], xtkv_sb[kc][:, tb2 * 128:(tb2 + 1) * 128],
                        wkva_sb[kc][:, :RKV], start=(kc == 0),
                        stop=(kc == KC - 1))
                for kc in range(KC):
                    nc.tensor.matmul(
                        pr[:], xtkv_sb[kc][:, tb2 * 128:(tb2 + 1) * 128],
                        wkva_sb[kc][:, RKV:], start=(kc == 0),
                        stop=(kc == KC - 1))
                st = gs.tile([128, RKV + DR], BF16, tag="kvst")
                nc.scalar.copy(st[:, :RKV], ps[:])
                nc.vector.tensor_copy(st[:, RKV:], pr[:])
                nc.sync.dma_start(kvag_in[tb2 * 128:(tb2 + 1) * 128, :], st[:])
        nc.gpsimd.collective_compute(
            "AllGather", mybir.AluOpType.bypass,
            replica_groups=[[0, 1, 2, 3], [4, 5, 6, 7]],
            ins=[kvag_in.opt()], outs=[kvag_out.opt()])

    # read the AllReduce result back immediately (deferred loads may queue
    # behind it — they aren't needed until Q/KB/O)
    p_rq = ctx.enter_context(tc.tile_pool(name="p_rq", bufs=1))
    r1 = p_rq.tile([1, S], F32, tag="r1")
    nc.sync.dma_start(r1[:], ssq_out[:].rearrange("(one s) -> one s", one=1))

    wkbk_sb = load(wkbk, RC, "wkbk")
    wkbv_sb = load(wkbv, RC, "wkbv")
    cost2_sb = load(cost2, 1, "cost2")[0]
    sint2_sb = load(sint2, 1, "sint2")[0]
    wout_sb = load(wout, HPG, "wout")

    # ---------------- phase KV: kv_a, rmsnorm, rope(k), transposes ----------
    # No Pool-engine work here: the AllReduce above occupies the Pool queue
    # and would stall drains behind the inter-core barrier.
    # k-rope kept in TWO half-padded d-major tiles so the score matmuls can
    # pair head-even (qr rows 0-63) / head-odd (rows 64-127) operands with a
    # common base partition of 0.
    krT_lo = p_mid.tile([128, S], BF16, tag="krTl")
    krT_hi = p_mid.tile([128, S], BF16, tag="krTh")
    kvnT = [p_mid.tile([128, S], BF16, tag=f"kvn{rc}", name=f"kvn{rc}")
            for rc in range(RC)]
    # double-buffered [tok, 128] staging for kr with permanently-zero halves
    kr_lo = [p_mid.tile([128, 128], BF16, tag=f"krlo{i}", name=f"krlo{i}")
             for i in range(2)]
    kr_hi = [p_mid.tile([128, 128], BF16, tag=f"krhi{i}", name=f"krhi{i}")
             for i in range(2)]
    for i in range(2):
        nc.vector.memset(kr_lo[i][:, 64:], 0.0)
        nc.vector.memset(kr_hi[i][:, :64], 0.0)
    rq_bc = p_mid.tile([128, S], F32, tag="rq")
    p_rqs = ctx.enter_context(tc.tile_pool(name="p_rqs", bufs=1))

    def emit_rq_chain(rqp):
        # broadcast 1/rms_q over partitions; emitted mid-KV so its Act/DVE
        # ops land mid-queue and rq_bc is ready before phase Q's drains
        r2 = p_rqs.tile([1, S], F32, tag="r2")
        nc.scalar.activation(r2[:], r1[:], AF.Sqrt, scale=1.0 / RQ,
                             bias=epsc[0:1, 0:1])
        r3 = p_rqs.tile([1, S], F32R, tag="r3")
        with nc.allow_low_precision(reason="f32r broadcast operand"):
            nc.vector.reciprocal(r3[:], r2[:])
        for sp in range(NSP):
            pb = rqp.tile([128, 512], F32, tag="pb")
            nc.tensor.matmul(pb[:], ones_r[:], r3[:, sp * 512:(sp + 1) * 512],
                             start=True, stop=True)
            nc.scalar.copy(rq_bc[:, sp * 512:(sp + 1) * 512], pb[:])

    with tc.tile_pool(name="kp", bufs=2, space="PSUM") as kp, \
         tc.tile_pool(name="kp2", bufs=2, space="PSUM") as kp2, \
         tc.tile_pool(name="kpt", bufs=2, space="PSUM") as kpt, \
         tc.tile_pool(name="rqp", bufs=2, space="PSUM") as rqp, \
         tc.tile_pool(name="kin", bufs=3) as kin, \
         tc.tile_pool(name="ks", bufs=2) as ks:
        for tb in range(NT):
            if tb == NT // 2:
                emit_rq_chain(rqp)
            if use_kv_ag:
                kva = kin.tile([128, RKV + DR], BF16, tag="kva")
                nc.sync.dma_start(kva[:], kvag_out[tb * 128:(tb + 1) * 128, :])
                ps, pr = kva[:, :RKV], kva[:, RKV:]
            else:
                ps = kp.tile([128, RKV], F32, tag="kva")
                pr = kp2.tile([128, DR], F32, tag="kvr")
                for kc in range(KC):
                    nc.tensor.matmul(ps[:],
                                     x_sb[kc][:, tb * 128:(tb + 1) * 128],
                                     wkva_sb[kc][:, :RKV], start=(kc == 0),
                                     stop=(kc == KC - 1))
                for kc in range(KC):
                    nc.tensor.matmul(pr[:],
                                     x_sb[kc][:, tb * 128:(tb + 1) * 128],
                                     wkva_sb[kc][:, RKV:], start=(kc == 0),
                                     stop=(kc == KC - 1))
            # rmsnorm stats straight off PSUM (or the gathered tile)
            scr = ks.tile([128, RKV], BF16, tag="scr")
            ssq = ks.tile([128, 1], F32, tag="ssq")
            nc.scalar.activation(scr[:], ps[:], AF.Square, accum_out=ssq[:])
            rk = ks.tile([128, 1], F32, tag="rk")
            nc.scalar.activation(rk[:], ssq[:], AF.Sqrt, scale=1.0 / RKV,
                                 bias=epsc[:, 0:1])
            nc.vector.reciprocal(ssq[:], rk[:])
            kvn = ks.tile([128, RKV], BF16, tag="kvn")
            nc.scalar.activation(kvn[:], ps[:], AF.Copy, scale=ssq[:, 0:1])
            # rope on k_rope (token-major)
            co = cosk_sb[:, tb * DR:(tb + 1) * DR]
            si = sink_sb[:, tb * DR:(tb + 1) * DR]
            t1 = ks.tile([128, DR], F32, tag="t1")
            nc.vector.tensor_mul(t1[:, 0::2], pr[:, 1::2], si[:, 0::2])
            nc.vector.tensor_mul(t1[:, 1::2], pr[:, 0::2], si[:, 1::2])
            t2 = ks.tile([128, DR], F32, tag="t2")
            nc.vector.tensor_mul(t2[:], pr[:], co)
            klo, khi = kr_lo[tb % 2], kr_hi[tb % 2]
            nc.vector.tensor_add(klo[:, :64], t1[:], t2[:])
            nc.vector.tensor_add(khi[:, 64:], t1[:], t2[:])
            # transposes (drained on Act/DVE; Pool has no PSUM port)
            ptl = kpt.tile([128, 128], BF16, tag="pt")
            nc.tensor.transpose(ptl[:], klo[:], ident[:])
            nc.scalar.copy(krT_lo[:, tb * 128:(tb + 1) * 128], ptl[:])
            pth = kpt.tile([128, 128], BF16, tag="pt")
            nc.tensor.transpose(pth[:], khi[:], ident[:])
            nc.vector.tensor_copy(krT_hi[:, tb * 128:(tb + 1) * 128], pth[:])
            for rc in range(RC):
                pt2 = kpt.tile([128, 128], BF16, tag="pt")
                nc.tensor.transpose(pt2[:], kvn[:, rc * 128:(rc + 1) * 128],
                                    ident[:])
                if rc % 2 == 0:
                    nc.scalar.copy(kvnT[rc][:, tb * 128:(tb + 1) * 128], pt2[:])
                else:
                    nc.vector.tensor_copy(kvnT[rc][:, tb * 128:(tb + 1) * 128],
                                          pt2[:])

    # ---------------- phase Q: q_nope + q_rope (both d-major) ---------------
    # rms scaling fused into the PSUM drains via rq_bc.
    qnS = [p_mid.tile([128, S], BF16, tag=f"qnS{h}", name=f"qnS{h}")
           for h in range(HPG)]
    qrS = [p_mid.tile([128, S], BF16, tag=f"qrS{hp}", name=f"qrS{hp}")
           for hp in range(HPG // 2)]
    with tc.tile_pool(name="qp", bufs=2, space="PSUM") as qp, \
         tc.tile_pool(name="qp2", bufs=2, space="PSUM") as qp2, \
         tc.tile_pool(name="qs", bufs=3) as qs:
        for h in range(HPG):
            for sp in range(NSP):
                ps = qp.tile([128, 512], F32, tag="qn")
                for kc in range(KC):
                    nc.tensor.matmul(
                        ps[:], wqn_sb[kc][:, h * DN:(h + 1) * DN],
                        x_sb[kc][:, sp * 512:(sp + 1) * 512],
                        start=(kc == 0), stop=(kc == KC - 1))
                nc.vector.tensor_mul(qnS[h][:, sp * 512:(sp + 1) * 512],
                                     ps[:], rq_bc[:, sp * 512:(sp + 1) * 512])
        # q_rope: head pairs stacked on partitions (rows 0-63 / 64-127)
        for hp in range(HPG // 2):
            for sp in range(NSP):
                sl = slice(sp * 512, (sp + 1) * 512)
                ps = qp.tile([128, 512], F32, tag="qr")
                for kc in range(KC):
                    nc.tensor.matmul(
                        ps[:], wqr_sb[kc][:, hp * 128:(hp + 1) * 128],
                        x_sb[kc][:, sl], start=(kc == 0), stop=(kc == KC - 1))
                # scale by 1/rms first (commutes with rope), then rotate
                qr0 = qs.tile([128, 512], BF16, tag="qr0")
                nc.vector.tensor_mul(qr0[:], ps[:], rq_bc[:, sl])
                ps2 = qp2.tile([128, 512], F32, tag="shuf")
                nc.tensor.matmul(ps2[:], perm_sb[:], qr0[:],
                                 start=True, stop=True)
                t1 = qs.tile([128, 512], F32, tag="t1")
                nc.vector.tensor_mul(t1[:], qr0[:], cost2_sb[:, sl])
                t2 = qs.tile([128, 512], F32, tag="t2")
                nc.vector.tensor_mul(t2[:], ps2[:], sint2_sb[:, sl])
                nc.vector.tensor_add(qrS[hp][:, sl], t1[:], t2[:])

    # ---------------- phase KB: k_nope (d-major) + v (token-major) ----------
    knT = [p_mid.tile([128, S], BF16, tag=f"kn{h}", name=f"kn{h}")
           for h in range(HPG)]
    v_sb = [p_mid.tile([128, HPG * DV], BF16, tag=f"v{tb}", name=f"v{tb}")
            for tb in range(NT)]
    with tc.tile_pool(name="bp", bufs=2, space="PSUM") as bp:
        for h in range(HPG):
            for sp in range(NSP):
                ps = bp.tile([128, 512], F32, tag="kn")
                for rc in range(RC):
                    nc.tensor.matmul(
                        ps[:], wkbk_sb[rc][:, h * DN:(h + 1) * DN],
                        kvnT[rc][:, sp * 512:(sp + 1) * 512],
                        start=(rc == 0), stop=(rc == RC - 1))
                if (h + sp) % 2 == 0:
                    nc.scalar.copy(knT[h][:, sp * 512:(sp + 1) * 512], ps[:])
                else:
                    nc.vector.tensor_copy(knT[h][:, sp * 512:(sp + 1) * 512],
                                          ps[:])
        for tb in range(NT):
            ps = bp.tile([128, HPG * DV], F32, tag="v")
            for rc in range(RC):
                nc.tensor.matmul(ps[:], kvnT[rc][:, tb * 128:(tb + 1) * 128],
                                 wkbv_sb[rc][:], start=(rc == 0),
                                 stop=(rc == RC - 1))
            if tb % 2 == 0:
                nc.scalar.copy(v_sb[tb][:], ps[:])
            else:
                nc.vector.tensor_copy(v_sb[tb][:], ps[:])

    # ------- phase A+O: attention (transposed flash, exact widths) + out ----
    attnT = [p_mid.tile([128, S], BF16, tag=f"at{h}", name=f"at{h}")
             for h in range(HPG)]
    with tc.tile_pool(name="op", bufs=2, space="PSUM") as op_, \
         tc.tile_pool(name="ap", bufs=2, space="PSUM") as ap_, \
         tc.tile_pool(name="sp", bufs=2, space="PSUM") as sp_, \
         tc.tile_pool(name="dp", bufs=2, space="PSUM") as dp, \
         tc.tile_pool(name="as_", bufs=3) as as_, \
         tc.tile_pool(name="os_", bufs=4) as os_:
        for sp in range(NSP):
            q0 = sp * 512
            nkb = 4 * (sp + 1)
            for h in range(HPG):
                acc = ap_.tile([128, 512], F32, tag="acc")
                den = dp.tile([1, 512], F32, tag="dn")
                for kb in range(nkb):
                    off = max(0, kb * 128 - q0)   # first visible col in span
                    ps = sp_.tile([128, 512], F32, tag="s")
                    nc.tensor.matmul(ps[:, off:],
                                     knT[h][:, kb * 128:(kb + 1) * 128],
                                     qnS[h][:, q0 + off:q0 + 512],
                                     start=True, stop=False)
                    krT_h = krT_lo if h % 2 == 0 else krT_hi
                    nc.tensor.matmul(ps[:, off:],
                                     krT_h[:, kb * 128:(kb + 1) * 128],
                                     qrS[h // 2][:, q0 + off:q0 + 512],
                                     start=False, stop=True)
                    e = as_.tile([128, 512], BF16, tag="e")
                    nc.scalar.activation(e[:, off:], ps[:, off:], AF.Exp,
                                         scale=SCALE)
                    if kb * 128 >= q0:
                        # diagonal block: triangle on the first visible 128
                        # (Pool: SBUF-only op, keeps the DVE queue clear)
                        nc.gpsimd.tensor_mul(e[:, off:off + 128],
                                             e[:, off:off + 128], tri_sb[:])
                    # exact-width accumulation: cols [0,off) finished their
                    # sums in earlier kb blocks; start zeroes the whole bank
                    # at kb=0 (off=0) and stop ends the bank's group
                    st, fin = (kb == 0), (kb == nkb - 1)
                    nc.tensor.matmul(acc[:, off:],
                                     v_sb[kb][:, h * DV:(h + 1) * DV],
                                     e[:, off:], start=st, stop=fin)
                    nc.tensor.matmul(den[:, off:], ones[:, 0:1], e[:, off:],
                                     start=st, stop=fin)
                rd = as_.tile([1, 512], F32R, tag="rd", bufs=2)
                with nc.allow_low_precision(reason="f32r broadcast operand"):
                    nc.vector.reciprocal(rd[:], den[:])
                rdp = dp.tile([128, 512], F32, tag="dn")
                nc.tensor.matmul(rdp[:], ones_r[:], rd[:], start=True,
                                 stop=True)
                rdb = as_.tile([128, 512], F32, tag="rdb", bufs=2)
                nc.vector.tensor_copy(rdb[:], rdp[:])
                nc.vector.tensor_mul(attnT[h][:, q0:q0 + 512], acc[:], rdb[:])
            # output projection for this span
            for mb in range(KC):
                po = op_.tile([128, 512], F32, tag="o")
                for hc in range(HPG):
                    nc.tensor.matmul(po[:],
                                     wout_sb[hc][:, mb * 128:(mb + 1) * 128],
                                     attnT[hc][:, q0:q0 + 512],
                                     start=(hc == 0), stop=(hc == HPG - 1))
                ot = os_.tile([128, 512], BF16, tag="ot")
                # alternate drain engines; the tail of the last span goes to
                # Act, which has drained its exp queue by then
                if mb % 2 == 0 or (sp == NSP - 1 and mb >= KC - 2):
                    nc.scalar.copy(ot[:], po[:])
                else:
                    nc.vector.tensor_copy(ot[:], po[:])
                # alternate output DMAs across the Pool and SP queues: one
                # queue alone can't sustain PE's drain production rate
                dma_eng = nc.gpsimd if mb % 2 == 0 else nc.sync
                dma_eng.dma_start(outT[:, mb * S + q0:mb * S + q0 + 512],
                                  ot[:])


def _fix_multiwait(nc):
    """This container's walrus only supports ONE sem-wait per instruction.
    Hoist excess waits onto freshly inserted same-engine Drain instructions
    placed immediately before the owner (engine executes in order, so the
    AND-semantics of multiple waits is preserved)."""
    import bass_rust
    n = [0]
    for fn in nc.m.functions:
        for blk in fn.blocks:
            out, changed = [], False
            for inst in blk.instructions:
                si = inst.sync_info
                waits = list(si.on_wait) if (si is not None and si.on_wait) else []
                if len(waits) > 1:
                    changed = True
                    for w in waits[:-1]:
                        n[0] += 1
                        d = bass_rust.InstDrain(
                            name=f"MWFIX-{n[0]}", engine=inst.engine,
                            ins=[], outs=[])
                        d.sync_info = bass_rust.SyncInfo(on_wait=[w],
                                                         on_update=[])
                        out.append(d)
                    si.on_wait = [waits[-1]]
                    inst.sync_info = si
                out.append(inst)
            if changed:
                blk.instructions = out


# ======================= host-side preparation =======================

def _bf16(a):
    return np.asarray(a, np.float32).astype(ml_dtypes.bfloat16)


def rope_tables():
    inv_freq = 1.0 / THETA ** (np.arange(0, DR, 2, dtype=np.float32) / DR)
    pos = np.arange(S, dtype=np.float32)
    freqs = np.outer(pos, inv_freq)
    emb = np.concatenate([freqs, freqs], axis=-1)          # [S, 64]
    cos = np.cos(emb).astype(np.float32)
    sin = np.sin(emb).astype(np.float32)
    sin_s = sin.copy()
    sin_s[:, 0::2] *= -1.0
    return cos, sin_s


def analyze_mask(mask):
    """mask: [1,1,S,S] additive causal. Returns block_cls (for the program
    cache key / interface compat) + the shared [128,128] triangular tile."""
    m = np.asarray(mask, np.float32).reshape(S, S)          # [q, k]
    block_cls = {}
    tri = None
    for qt in range(S // 256):
        for kb in range(NT):
            sub = m[qt * 256:(qt + 1) * 256, kb * 128:(kb + 1) * 128]
            if np.all(sub <= -1e8):
                block_cls[(kb, qt)] = SKIP
            elif np.all(sub == 0.0):
                block_cls[(kb, qt)] = FREE
            else:
                block_cls[(kb, qt)] = MIXED
    # the program assumes causal structure; verify and extract the diagonal
    # [k=128, q=128] visibility tile (identical for every diagonal block)
    for db in range(NT):
        sub = m[db * 128:(db + 1) * 128, db * 128:(db + 1) * 128]
        t = (sub.T > -1e8).astype(np.float32)               # [k, q]
        if tri is None:
            tri = t
        assert np.array_equal(tri, t), "non-uniform diagonal mask blocks"
    for qb in range(NT):
        for kb in range(NT):
            if kb == qb:
                continue
            sub = m[qb * 128:(qb + 1) * 128, kb * 128:(kb + 1) * 128]
            if kb < qb:
                assert np.all(sub == 0.0), "non-causal mask"
            else:
                assert np.all(sub <= -1e8), "non-causal mask"
    return block_cls, _bf16(tri)


def prep_core_inputs(inputs, wqa_cols=WQA_SL):
    """Returns (in_maps list of 8 dicts, block_cls)."""
    x = np.asarray(inputs["x"], np.float32)
    Wqa = np.asarray(inputs["Wqa"], np.float32)
    qw = np.asarray(inputs["q_a_norm_w"], np.float32)
    Wqb = np.asarray(inputs["Wqb"], np.float32)
    Wkva = np.asarray(inputs["Wkva"], np.float32)
    kvw = np.asarray(inputs["kv_a_norm_w"], np.float32)
    Wkvb = np.asarray(inputs["Wkvb"], np.float32)
    Wout = np.asarray(inputs["Wout"], np.float32)

    block_cls, tri = analyze_mask(inputs["attention_mask"])

    wq_eff = Wqa @ (qw[:, None] * Wqb)                      # [D, H*192]
    wq_eff = wq_eff.reshape(D, H, DN + DR)
    wkvb_w = kvw[:, None] * Wkvb                            # [RKV, H*256]
    wkvb_w = wkvb_w.reshape(RKV, H, DN + DV)
    wout_h = Wout.reshape(H, DV, D)

    cos, sin_s = rope_tables()                              # [S, 64]
    # token-major tables for k-rope: [128, NT*DR]
    cosk = _bf16(cos.reshape(NT, 128, DR).transpose(1, 0, 2).reshape(128, -1))
    sink = _bf16(sin_s.reshape(NT, 128, DR).transpose(1, 0, 2).reshape(128, -1))
    # dim-major stacked tables for q-rope: rows 0-63 and 64-127 identical
    cost2 = _bf16(np.concatenate([cos.T, cos.T], axis=0))   # [128, S]
    sint2 = _bf16(np.concatenate([sin_s.T, sin_s.T], axis=0))
    # pair-swap permutation (used as both rope shuffle and PE transpose aux)
    pm = np.zeros((128, 128), np.float32)
    for i in range(128):
        pm[i, i ^ 1] = 1.0
    perm = _bf16(pm)

    def pack(a):
        """[C*128, W] -> [128, C*W] chunk-packed bf16."""
        rows, w = a.shape
        c = rows // 128
        return _bf16(a.reshape(c, 128, w).transpose(1, 0, 2).reshape(128, -1))

    in_maps = []
    for c in range(NCORE):
        b, g = c // TP, c % TP
        hs = slice(g * HPG, (g + 1) * HPG)
        m = {
            "xT": pack(x[b].T),
            "xTkv": pack(x[b].T[:, g * (S // TP):(g + 1) * (S // TP)]),
            "wqa": pack(Wqa[:, g * wqa_cols:(g + 1) * wqa_cols]
                        if wqa_cols < RQ else Wqa),
            "wqn": pack(wq_eff[:, hs, :DN].reshape(D, HPG * DN)),
            "wqr": pack(wq_eff[:, hs, DN:].reshape(D, HPG * DR)),
            "wkva": pack(Wkva),
            "wkbk": pack(wkvb_w[:, hs, :DN].reshape(RKV, HPG * DN)),
            "wkbv": pack(wkvb_w[:, hs, DN:].reshape(RKV, HPG * DV)),
            "wout": pack(wout_h[hs].reshape(HPG * DV, D)),
            "cost2": cost2,
            "sint2": sint2,
            "cosk": cosk,
            "sink": sink,
            "perm": perm,
            "masks": tri,
        }
        in_maps.append(m)
    return in_maps, block_cls


def postprocess(results):
    """results: list of 8 dicts with 'outT' [128, KC*S] bf16 chunk-packed
    partials (outT[p, mb*S + s] = out^T[mb*128 + p, s])."""
    out = np.empty((B, S, D), np.float32)
    for b in range(B):
        acc = results[b * TP]["outT"].astype(np.float32)
        for g in range(1, TP):
            acc = acc + results[b * TP + g]["outT"].astype(np.float32)
        # [128, KC*S] -> [KC*128, S] = outT -> transpose to [S, D]
        acc = acc.reshape(128, KC, S).transpose(1, 0, 2).reshape(D, S)
        out[b] = acc.T
    return out


# ======================= kernel entry point =======================

_program_cache = {}


def _mask_key(block_cls, packed):
    h = hashlib.sha256()
    h.update(repr(sorted(block_cls.items())).encode())
    h.update(np.ascontiguousarray(packed).tobytes())
    return h.hexdigest()


def kernel(**inputs):
    """Full-input MLA forward on 8 NeuronCores.

    Sharding: data-parallel over batch (2) x tensor-parallel over heads
    (4 groups of 4); the per-token q-RMS statistic is AllReduce'd inside
    each batch group. Host folds Wqa@Wqb, shards weights by head, casts to
    bf16 and transposes x; device returns per-core transposed bf16 partial
    outputs which the host sums per batch group.
    """
    from concourse.bass_utils import run_bass_kernel_spmd

    in_maps, block_cls = prep_core_inputs(inputs)
    key = _mask_key(block_cls, in_maps[0]["masks"])
    nc = _program_cache.get(key)
    if nc is None:
        nc = build_program(block_cls, use_collective=True)
        _program_cache[key] = nc
    res = run_bass_kernel_spmd(nc, in_maps, core_ids=list(range(NCORE)))
    return postprocess(res.results)
